# revision 53
# baseline (speedup 1.0000x reference)
"""Trainium2 Bass kernel for nn_BornFoward: 200-step leapfrog wave recurrence.

Math (validated against the jax reference in a numpy model):
  - coef = (dt*BGF/dx)^2 is 0.2025 in the interior square [25:167)^2 of the
    192x192 grid and ~4.4e-13 in the outer absorbing ring; rf is EXACTLY zero
    outside the central 96x96 window (pad region has X==1 -> 1-X^2==0).
  - Therefore the recurrence restricted to the 142x142 interior with zero
    Dirichlet boundary and constant coef reproduces the reference to ~1e-9.
  - p_new = 2*p1 - p0 + C*lap4(p1) + rf*d2(P0),  meas = p_new at 32 pixels.

Sharding: 16 independent recurrences (B=2 x NR=8) -> channel r per core,
both batches per core, batched along the matmul free (column) dimension.

Layout per core: state tiles [71 partitions, 2 chunks x 292], where each
chunk holds rows 71k..71k+70 as two field segments [2 guard | 142 | 2 guard].
All matmul rhs/out operands are contiguous 288-column windows ([2,290) of
the run; N=288 >= 256 so float32r matmuls stream at 1 cycle/row, ~120ns).

Per-core per-step compute (engine assignment balances the 5 engines):
  PE (12 matmuls):
    PSUM_m = band_x @ p1        (x-stencil + diag + 2I; 2 K-chunks; +32 meas
                                 selection rows augmented onto chunk-0 lhsT)
           + a*I @ p1(cols+-1) + b*I @ p1(cols+-2)  (y-stencil, shifted rhs)
  GpSimd:  p0 -= G[j]           (presubtract on the central cols; p0 is dead
                                 as a matmul operand -> a full step of slack)
  DVE:     p_new = PSUM_m - p0  (fused copyback, rotates state tiles)
  Act:     stages the 32 PSUM selection rows into SBUF
  DVE:     per-field one-hot mask-reduce STTs (SBUF-sourced, 208ns each)

The G term enters via p_new = PSUM - (p0 - G), eliminating 2 of the 14
matmuls per step. Steady state ~1715ns/step: bounded not by PE occupancy
(84%) but by the recurrence-carried chain  ps-close -> sem -> DVE copyback
(421ns) -> ack+sem -> next step's matmuls  (~924ns of which ~520 is fixed
sem/ack latency). Only DVE can do PSUM-elementwise ops, matmul rhs must be
SBUF, and engine free-size costing makes tiny-partition fix-ups cost as
much as full ones, so this chain is structural; see session notes.

Startup: one packed-constants DMA + engine memsets (each DMA costs ~630ns
of HWDGE queue time regardless of size), first matmul at ~3.4us.
"""
import sys
import os
import numpy as np
from contextlib import ExitStack

sys.path.insert(0, "/opt/trn_rl_repo")

# ---- problem constants (hardcoded; kernel.py must be self-contained) ----
NX = 192
NT = 200
dtime = 0.3
nm, sR = 32, 70
bg = 1.5
LO, HI = 25, 167            # interior rows/cols [LO, HI) -> D = 142
D = HI - LO
CLO, CHI = 48, 144          # central rf-support window (96 wide)
CW = CHI - CLO
COFF = CLO - LO             # 23: central window offset inside domain
C = (dtime * bg / 1.0) ** 2  # 0.2025
K = 71                      # row-chunk size (2 chunks of 71 = 142)
SEG = 2 + D + 2             # 146: per-field segment with 2-col guards
CHW = 2 * SEG               # 292: chunk width (two fields)
NRR = 8
BB = 2
NMEAS = nm

_thetas = 2 * np.pi * np.arange(nm) / nm
_MX = (NX / 2 + sR * np.cos(_thetas)).astype(int)
_MY = (NX / 2 + sR * np.sin(_thetas)).astype(int)

INCLUDE_2I = True           # fold the 2*p1 term into the band matmul

_prog_cache = {}


def _build_band_consts():
    """Host-side constant matrices for the matmuls (numpy float32)."""
    S = np.zeros((D, D), np.float32)
    idx = np.arange(D)
    S[idx, idx] = -60.0 * C / 12.0 + (2.0 if INCLUDE_2I else 0.0)
    S[idx[:-1], idx[:-1] + 1] = 16.0 * C / 12.0
    S[idx[1:], idx[1:] - 1] = 16.0 * C / 12.0
    S[idx[:-2], idx[:-2] + 2] = -C / 12.0
    S[idx[2:], idx[2:] - 2] = -C / 12.0

    BD = {}
    for kc in range(2):
        for mc in range(2):
            blk = S[mc * K:(mc + 1) * K, kc * K:(kc + 1) * K].T.copy()
            if mc == 0:
                aug = np.zeros((K, 96 + NMEAS), np.float32)
                aug[:, :K] = blk
                for i in range(NMEAS):
                    g = _MX[i] - LO
                    if g // K == kc:
                        aug[g % K, 96 + i] = 1.0
                blk = aug
            BD[(kc, mc)] = np.ascontiguousarray(blk)

    SH1 = np.eye(K, dtype=np.float32) * np.float32(16.0 * C / 12.0)
    SH2 = np.eye(K, dtype=np.float32) * np.float32(-C / 12.0)

    # per-field one-hot masks over the 142 data cols
    MASK = np.zeros((NMEAS, D), np.float32)
    for i in range(NMEAS):
        MASK[i, _MY[i] - LO] = 1.0
    return BD, SH1, SH2, MASK


def _build_program(nt=NT, debug=False, reps=1):
    import concourse.bacc as bacc
    import concourse.tile as tile
    import concourse.mybir as mybir

    dt = mybir.dt
    nc = bacc.Bacc("TRN2", target_bir_lowering=False)

    G_d = nc.dram_tensor("G", (NT, BB, CW, CW), dt.float32, kind="ExternalInput")
    # all small constants packed into one tensor -> single startup DMA
    # layout (cols): bd00[128] bd01[128] bd10[71] bd11[71] sh1[71] sh2[71]
    #                mask[142] = 682
    CPACK = 128 + 128 + K + K + K + K + D
    CONST_d = nc.dram_tensor("CONSTS", (K, CPACK), dt.float32r,
                             kind="ExternalInput")
    OUT_d = nc.dram_tensor("OUT", (BB, NMEAS, NT), dt.float32, kind="ExternalOutput")
    if debug:
        DBGC_d = nc.dram_tensor("DBGC", (2, K, 300), dt.float32, kind="ExternalOutput")
        DBGP_d = nc.dram_tensor("DBGP", (2, K, 300), dt.float32, kind="ExternalOutput")

    GPF = 4  # G stream ring depth
    PAD = 4  # left/right pad so shift offsets stay in-bounds

    with tile.TileContext(nc) as tc, ExitStack() as ctx:
        def sbuf(name, shape, dty):
            return ctx.enter_context(nc.sbuf_tensor(name, shape, dty))

        # per-chunk state tiles: [4 pad | 2 x (2+142+2) | 4 pad] = 300 cols
        PA = [sbuf(f"PA{kc}", [K, 300], dt.float32r) for kc in range(2)]
        PB = [sbuf(f"PB{kc}", [K, 300], dt.float32r) for kc in range(2)]
        # G ring: compact central-cols tiles per chunk; partitions outside
        # the central rows stay zero (engine ops need 32-aligned partition
        # bases, so the presub spans all 71 chunk rows)
        Gr = [[sbuf(f"Gr{i}_{kc}", [K, BB * CW], dt.float32) for kc in range(2)]
              for i in range(GPF)]
        const_t = sbuf("consts", [K, CPACK], dt.float32r)
        _off = [0]

        def cslice(w, parts=K, cast=None):
            lo = _off[0]
            _off[0] += w
            v = const_t[0:parts, lo:lo + w]
            return v.bitcast(cast) if cast is not None else v

        # pack order groups the consts needed by the first 7 matmul slots
        # (bd00, bd10, bd01, sh1, sh2) ahead of the late ones (bd11, mask)
        # so the startup DMA can split and un-gate the first step earlier
        bd_t = {(0, 0): cslice(128), (1, 0): cslice(128), (0, 1): cslice(K)}
        sh1_t = cslice(K)
        sh2_t = cslice(K)
        CSPLIT = _off[0]                                  # 469
        bd_t[(1, 1)] = cslice(K)
        mask_t = cslice(D, parts=NMEAS, cast=dt.float32)
        meas_t = sbuf("meas", [NMEAS, BB * NT], dt.float32)
        scr_t = [sbuf(f"scr{f}", [NMEAS, D], dt.float32) for f in range(2)]
        # staging for the PSUM aug rows (Act copies here, STTs read SBUF)
        augs_t = [sbuf(f"augs{i}", [NMEAS, 2 * D], dt.float32)
                  for i in range(2)]

        ps_pool = ctx.enter_context(tc.tile_pool(name="ps", bufs=3, space="PSUM"))

        # constants in two DMAs (early-gating slice first); zero-inits via
        # engine memsets (each startup DMA costs ~630ns of HWDGE queue time
        # regardless of size)
        nc.sync.dma_start(const_t[:, 0:CSPLIT], CONST_d[:, 0:CSPLIT])
        nc.sync.dma_start(const_t[:, CSPLIT:], CONST_d[:, CSPLIT:])
        for kc in range(2):
            nc.vector.memset(PA[kc][:].bitcast(dt.float32), 0.0)
            nc.vector.memset(PB[kc][:].bitcast(dt.float32), 0.0)
        for i in range(GPF):
            nc.gpsimd.memset(Gr[i][0][:], 0.0)
            nc.gpsimd.memset(Gr[i][1][:], 0.0)
        nc.vector.memset(meas_t[:], 0.0)

        def g_dma(j):
            """DMA G[j] (BB, 96, 96) into ring slot j%GPF (central rows)."""
            for kc in range(2):
                gt = Gr[j % GPF][kc]
                plo = COFF if kc == 0 else 0          # central partition base
                rlo = 0 if kc == 0 else 48            # central row base
                src = G_d[j, :, rlo:rlo + 48, :].rearrange("f r c -> r f c")
                dst = gt[plo:plo + 48, :].rearrange("p (f c) -> p f c", c=CW)
                # Activation's HWDGE queue: keeps the G stream off the SP
                # queue so startup consts and G prefetch flow in parallel
                nc.scalar.dma_start(dst, src)

        def g_presub(j, prev):
            """prev (old state, p0) -= G[j] on the central cols (GpSimd)."""
            for kc in range(2):
                base = prev[kc][:, PAD:PAD + CHW]

                def cvi(b):
                    return b.rearrange("p (f c) -> p f c", c=SEG)[
                        :, :, 2 + COFF:2 + COFF + CW]
                gv = Gr[j % GPF][kc][:].rearrange("p (f c) -> p f c", c=CW)
                nc.gpsimd.tensor_sub(
                    cvi(base), cvi(base.bitcast(dt.float32)), gv)

        # matmul operands use the strided 2-field data view: free size
        # 2x142 = 284 still clears the fp32r N>=256 threshold, so each
        # matmul streams 284 columns instead of a contiguous 288 window
        # (cost model prices output free-size; guards are never written)
        def run_view(t, off=0):
            """Strided [71, 2, 142] matmul-rhs data view at col-tap off."""
            v = t[:, PAD + off: PAD + off + CHW]
            return v.rearrange("p (f c) -> p f c", c=SEG)[:, :, 2:2 + D]

        def data_view(t, cast_f32=False):
            """[71, 2(field), 142] data view (for DVE ops)."""
            v = t[:, PAD:PAD + CHW]
            if cast_f32:
                v = v.bitcast(dt.float32)
            return v.rearrange("p (f c) -> p f c", c=SEG)[:, :, 2:2 + D]

        def central_view(t, cast_f32=False):
            """[71, 2(field), 96] central-cols view of a state chunk tile."""
            v = t[:, PAD:PAD + CHW]
            if cast_f32:
                v = v.bitcast(dt.float32)
            return v.rearrange("p (f c) -> p f c", c=SEG)[
                :, :, 2 + COFF:2 + COFF + CW]

        def meas_extract(pt, j, direct=False):
            """Extract 32x2 measurements for output step j from selection rows.

            Act stages the PSUM aug rows into SBUF (keeps the PSUM read off
            DVE); the per-field mask-reduce STTs then run on DVE. The
            epilogue uses direct=True (STT straight from PSUM) since the
            staging hop only adds serial latency there."""
            if direct:
                for f in range(2):
                    seg = pt[96:96 + NMEAS, f * SEG + 2: f * SEG + 2 + D]
                    nc.vector.scalar_tensor_tensor(
                        out=scr_t[f][:], in0=seg, scalar=1.0, in1=mask_t,
                        op0=mybir.AluOpType.mult, op1=mybir.AluOpType.mult,
                        accum_out=meas_t[:, f * NT + j: f * NT + j + 1],
                    )
                return
            at = augs_t[j % 2]
            src = pt[96:96 + NMEAS, :].rearrange(
                "p (f c) -> p f c", c=SEG)[:, :, 2:2 + D]
            nc.scalar.copy(at[:].rearrange("p (f c) -> p f c", c=D), src)
            for f, eng in ((0, nc.vector), (1, nc.vector)):
                eng.scalar_tensor_tensor(
                    out=scr_t[f][:], in0=at[:, f * D:(f + 1) * D], scalar=1.0,
                    in1=mask_t,
                    op0=mybir.AluOpType.mult, op1=mybir.AluOpType.mult,
                    accum_out=meas_t[:, f * NT + j: f * NT + j + 1],
                )

        cur, prev = PA, PB
        for rep in range(reps):
          if rep > 0:
            # re-zero state so values stay bounded across timing reps
            for kc in range(2):
                nc.vector.memset(PA[kc][:].bitcast(dt.float32), 0.0)
                nc.vector.memset(PB[kc][:].bitcast(dt.float32), 0.0)
          # iters 0,1 are skipped: G[0]=G[1]=0 (host zeroes d2[:, :2]) and
          # the state starts at zero, so p_1 = p_2 = 0 and output cols 0,1
          # are exactly zero -- provided by the meas_t memset.
          J0 = 2
          for j in range(J0, nt):
              if j == J0:
                  for q in range(J0, min(J0 + GPF - 1, nt)):
                      g_dma(q)
              if j + GPF - 1 < nt:
                  g_dma(j + GPF - 1)

              g_presub(j, prev)

              if j == J0:
                  # state is still all-zero at the first computed iter, so
                  # every matmul would produce zeros and p_{J0+1} = G_{J0}
                  # exactly: realize it as cur = -(0 - G) on DVE and skip
                  # the 12 matmuls and their copyback chain entirely.
                  # (output col J0-1 = 0 is covered by the meas_t memset.)
                  for mc in range(2):
                      nc.vector.tensor_scalar_mul(
                          data_view(prev[mc]),
                          data_view(prev[mc], cast_f32=True), -1.0)
                  cur, prev = prev, cur
                  continue

              # emission order: all cur[0]-gated matmuls first (ps0 body +
              # ps1's bd01 start), then the cur[1]-gated tail; bd10 closes
              # ps0 as late as its dependency allows so copyback0 tracks
              # copyback1's phase.
              pt0 = ps_pool.tile([128, CHW], dt.float32, tag="ps0")
              pt1 = ps_pool.tile([K, CHW], dt.float32, tag="ps1")

              def dv(ap):
                  return ap.rearrange("p (f c) -> p f c", c=SEG)[:, :, 2:2 + D]
              pd0, pd1 = dv(pt0[0:K, :]), dv(pt1[0:K, :])
              full0 = dv(pt0[:])
              nc.tensor.matmul(full0, bd_t[(0, 0)], run_view(cur[0]),
                               start=True, stop=False)
              for s, sh in ((-1, sh1_t), (1, sh1_t), (-2, sh2_t), (2, sh2_t)):
                  nc.tensor.matmul(pd0, sh, run_view(cur[0], s),
                                   start=False, stop=False)
              nc.tensor.matmul(pd1, bd_t[(0, 1)], run_view(cur[0]),
                               start=True, stop=False)
              nc.tensor.matmul(full0, bd_t[(1, 0)], run_view(cur[1]),
                               start=False, stop=True)
              for s, sh in ((-1, sh1_t), (1, sh1_t), (-2, sh2_t), (2, sh2_t)):
                  nc.tensor.matmul(pd1, sh, run_view(cur[1], s),
                                   start=False, stop=False)
              nc.tensor.matmul(pd1, bd_t[(1, 1)], run_view(cur[1]),
                               start=False, stop=True)
              psums = [pt0, pt1]

              for mc in range(2):
                  pd = psums[mc][0:K, :].rearrange(
                      "p (f c) -> p f c", c=SEG)[:, :, 2:2 + D]
                  nc.vector.tensor_tensor(
                      out=data_view(prev[mc]), in0=pd,
                      in1=data_view(prev[mc], cast_f32=True),
                      op=mybir.AluOpType.subtract)

              if j > 0:
                  meas_extract(psums[0], j - 1, direct=(j == nt - 1))

              cur, prev = prev, cur

        # final measurement for output step nt-1 on the final state
        pt = ps_pool.tile([128, CHW], dt.float32, tag="ps0")
        fdv = pt[:].rearrange("p (f c) -> p f c", c=SEG)[:, :, 2:2 + D]
        nc.tensor.matmul(fdv, bd_t[(0, 0)], run_view(cur[0]),
                         start=True, stop=False)
        nc.tensor.matmul(fdv, bd_t[(1, 0)], run_view(cur[1]),
                         start=False, stop=True)
        meas_extract(pt, nt - 1, direct=True)

        if debug:
            for kc in range(2):
                nc.sync.dma_start(DBGC_d[kc], cur[kc][:].bitcast(dt.float32))
                nc.sync.dma_start(DBGP_d[kc], prev[kc][:].bitcast(dt.float32))
        nc.sync.dma_start(
            OUT_d[:].rearrange("f i j -> i f j"),
            meas_t[:].rearrange("i (f j) -> i f j", j=NT))

    nc.compile()
    return nc


def kernel(x, P0):
    x = np.asarray(x, dtype=np.float32)
    P0 = np.asarray(P0, dtype=np.float32)
    from concourse.bass_utils import run_bass_kernel_spmd

    if "prog" not in _prog_cache:
        _prog_cache["prog"] = _build_program()
    nc = _prog_cache["prog"]

    BD, SH1, SH2, MASK = _build_band_consts()

    xx = bg / x[:, 0]
    rf = (1.0 - xx * xx).astype(np.float32)           # (B, 96, 96)
    P0c = P0[0, :, :, CLO:CHI, CLO:CHI]               # (NR, NT, 96, 96)
    d2 = np.zeros_like(P0c)
    d2[:, 2:] = P0c[:, 2:] - 2.0 * P0c[:, 1:-1] + P0c[:, :-2]

    # pack constants in the kernel's cslice order:
    # bd00[128] bd10[128] bd01[71] sh1[71] sh2[71] | bd11[71] mask[142]
    CPACK = 128 + 128 + K + K + K + K + D
    cp = np.zeros((K, CPACK), np.float32)
    off = 0
    for blk in (BD[(0, 0)], BD[(1, 0)], BD[(0, 1)], SH1, SH2, BD[(1, 1)]):
        w = blk.shape[1]
        cp[:, off:off + w] = blk
        off += w
    cp[:NMEAS, off:off + D] = MASK
    consts = {"CONSTS": cp}

    in_maps = []
    for r in range(NRR):
        G = (rf[None, :, :, :] * d2[r][:, None, :, :]).astype(np.float32)
        m = dict(consts)
        m["G"] = np.ascontiguousarray(G)
        in_maps.append(m)

    trace = bool(int(os.environ.get("KERNEL_TRACE", "0")))
    res = run_bass_kernel_spmd(nc, in_maps, core_ids=list(range(NRR)),
                               trace=trace)
    _prog_cache["last_result"] = res
    out = np.zeros((BB, NRR, NMEAS, NT), np.float32)
    for r in range(NRR):
        out[:, r] = res.results[r]["OUT"]
    return out



# revision 54
# speedup vs baseline: 1.0559x; 1.0559x over previous
"""Trainium2 Bass kernel for nn_BornFoward: 200-step leapfrog wave recurrence.

Math (validated against the jax reference in a numpy model):
  - coef = (dt*BGF/dx)^2 is 0.2025 in the interior square [25:167)^2 of the
    192x192 grid and ~4.4e-13 in the outer absorbing ring; rf is EXACTLY zero
    outside the central 96x96 window (pad region has X==1 -> 1-X^2==0).
  - Therefore the recurrence restricted to the 142x142 interior with zero
    Dirichlet boundary and constant coef reproduces the reference to ~1e-9.
  - p_new = 2*p1 - p0 + C*lap4(p1) + rf*d2(P0),  meas = p_new at 32 pixels.

Sharding: 16 independent recurrences (B=2 x NR=8) -> channel r per core,
both batches per core, batched along the matmul free (column) dimension.

Layout per core: state tiles [71 partitions, 2 chunks x 292], where each
chunk holds rows 71k..71k+70 as two field segments [2 guard | 142 | 2 guard].
All matmul rhs/out operands are contiguous 288-column windows ([2,290) of
the run; N=288 >= 256 so float32r matmuls stream at 1 cycle/row, ~120ns).

Per-core per-step compute (engine assignment balances the 5 engines):
  PE (12 matmuls):
    PSUM_m = band_x @ p1        (x-stencil + diag + 2I; 2 K-chunks; +32 meas
                                 selection rows augmented onto chunk-0 lhsT)
           + a*I @ p1(cols+-1) + b*I @ p1(cols+-2)  (y-stencil, shifted rhs)
  GpSimd:  p0 -= G[j]           (presubtract on the central cols; p0 is dead
                                 as a matmul operand -> a full step of slack)
  DVE:     p_new = PSUM_m - p0  (fused copyback, rotates state tiles)
  Act:     stages the 32 PSUM selection rows into SBUF
  DVE:     per-field one-hot mask-reduce STTs (SBUF-sourced, 208ns each)

The G term enters via p_new = PSUM - (p0 - G), eliminating 2 of the 14
matmuls per step. Steady state ~1715ns/step: bounded not by PE occupancy
(84%) but by the recurrence-carried chain  ps-close -> sem -> DVE copyback
(421ns) -> ack+sem -> next step's matmuls  (~924ns of which ~520 is fixed
sem/ack latency). Only DVE can do PSUM-elementwise ops, matmul rhs must be
SBUF, and engine free-size costing makes tiny-partition fix-ups cost as
much as full ones, so this chain is structural; see session notes.

Startup: one packed-constants DMA + engine memsets (each DMA costs ~630ns
of HWDGE queue time regardless of size), first matmul at ~3.4us.
"""
import sys
import os
import numpy as np
from contextlib import ExitStack

sys.path.insert(0, "/opt/trn_rl_repo")

# ---- problem constants (hardcoded; kernel.py must be self-contained) ----
NX = 192
NT = 200
dtime = 0.3
nm, sR = 32, 70
bg = 1.5
LO, HI = 25, 167            # interior rows/cols [LO, HI) -> D = 142
D = HI - LO
CLO, CHI = 48, 144          # central rf-support window (96 wide)
CW = CHI - CLO
COFF = CLO - LO             # 23: central window offset inside domain
C = (dtime * bg / 1.0) ** 2  # 0.2025
K = 71                      # row-chunk size (2 chunks of 71 = 142)
SEG = 2 + D + 2             # 146: per-field segment with 2-col guards
CHW = 2 * SEG               # 292: chunk width (two fields)
NRR = 8
BB = 2
NMEAS = nm

_thetas = 2 * np.pi * np.arange(nm) / nm
_MX = (NX / 2 + sR * np.cos(_thetas)).astype(int)
_MY = (NX / 2 + sR * np.sin(_thetas)).astype(int)

INCLUDE_2I = True           # fold the 2*p1 term into the band matmul

_prog_cache = {}


def _build_band_consts():
    """Host-side constant matrices for the matmuls (numpy float32)."""
    S = np.zeros((D, D), np.float32)
    idx = np.arange(D)
    S[idx, idx] = -60.0 * C / 12.0 + (2.0 if INCLUDE_2I else 0.0)
    S[idx[:-1], idx[:-1] + 1] = 16.0 * C / 12.0
    S[idx[1:], idx[1:] - 1] = 16.0 * C / 12.0
    S[idx[:-2], idx[:-2] + 2] = -C / 12.0
    S[idx[2:], idx[2:] - 2] = -C / 12.0

    BD = {}
    for kc in range(2):
        for mc in range(2):
            blk = S[mc * K:(mc + 1) * K, kc * K:(kc + 1) * K].T.copy()
            if mc == 0:
                aug = np.zeros((K, 96 + NMEAS), np.float32)
                aug[:, :K] = blk
                for i in range(NMEAS):
                    g = _MX[i] - LO
                    if g // K == kc:
                        aug[g % K, 96 + i] = 1.0
                blk = aug
            BD[(kc, mc)] = np.ascontiguousarray(blk)

    SH1 = np.eye(K, dtype=np.float32) * np.float32(16.0 * C / 12.0)
    SH2 = np.eye(K, dtype=np.float32) * np.float32(-C / 12.0)

    # per-field one-hot masks over the 142 data cols
    MASK = np.zeros((NMEAS, D), np.float32)
    for i in range(NMEAS):
        MASK[i, _MY[i] - LO] = 1.0
    return BD, SH1, SH2, MASK


def _build_program(nt=NT, debug=False, reps=1):
    import concourse.bacc as bacc
    import concourse.tile as tile
    import concourse.mybir as mybir

    dt = mybir.dt
    nc = bacc.Bacc("TRN2", target_bir_lowering=False)

    G_d = nc.dram_tensor("G", (NT, BB, CW, CW), dt.float32, kind="ExternalInput")
    # all small constants packed into one tensor -> single startup DMA
    # layout (cols): bd00[128] bd01[128] bd10[71] bd11[71] sh1[71] sh2[71]
    #                mask[142] = 682
    CPACK = 128 + 128 + K + K + K + K + D
    CONST_d = nc.dram_tensor("CONSTS", (K, CPACK), dt.float32r,
                             kind="ExternalInput")
    OUT_d = nc.dram_tensor("OUT", (BB, NMEAS, NT), dt.float32, kind="ExternalOutput")
    if debug:
        DBGC_d = nc.dram_tensor("DBGC", (2, K, 300), dt.float32, kind="ExternalOutput")
        DBGP_d = nc.dram_tensor("DBGP", (2, K, 300), dt.float32, kind="ExternalOutput")

    GPF = 4  # G stream ring depth
    PAD = 4  # left/right pad so shift offsets stay in-bounds

    with tile.TileContext(nc) as tc, ExitStack() as ctx:
        def sbuf(name, shape, dty):
            return ctx.enter_context(nc.sbuf_tensor(name, shape, dty))

        # per-chunk state tiles: [4 pad | 2 x (2+142+2) | 4 pad] = 300 cols
        PA = [sbuf(f"PA{kc}", [K, 300], dt.float32r) for kc in range(2)]
        PB = [sbuf(f"PB{kc}", [K, 300], dt.float32r) for kc in range(2)]
        # G ring: compact central-cols tiles per chunk; partitions outside
        # the central rows stay zero (engine ops need 32-aligned partition
        # bases, so the presub spans all 71 chunk rows)
        Gr = [[sbuf(f"Gr{i}_{kc}", [K, BB * CW], dt.float32) for kc in range(2)]
              for i in range(GPF)]
        const_t = sbuf("consts", [K, CPACK], dt.float32r)
        _off = [0]

        def cslice(w, parts=K, cast=None):
            lo = _off[0]
            _off[0] += w
            v = const_t[0:parts, lo:lo + w]
            return v.bitcast(cast) if cast is not None else v

        # pack order groups the consts needed by the first 7 matmul slots
        # (bd00, bd10, bd01, sh1, sh2) ahead of the late ones (bd11, mask)
        # so the startup DMA can split and un-gate the first step earlier
        bd_t = {(0, 0): cslice(128), (1, 0): cslice(128), (0, 1): cslice(K)}
        sh1_t = cslice(K)
        sh2_t = cslice(K)
        CSPLIT = _off[0]                                  # 469
        bd_t[(1, 1)] = cslice(K)
        mask_t = cslice(D, parts=NMEAS, cast=dt.float32)
        meas_t = sbuf("meas", [NMEAS, BB * NT], dt.float32)
        scr_t = [sbuf(f"scr{f}", [NMEAS, D], dt.float32) for f in range(2)]
        # staging for the PSUM aug rows (Act copies here, STTs read SBUF)
        augs_t = [sbuf(f"augs{i}", [NMEAS, 2 * D], dt.float32)
                  for i in range(2)]

        ps_pool = ctx.enter_context(tc.tile_pool(name="ps", bufs=3, space="PSUM"))

        # constants in two DMAs (early-gating slice first); zero-inits via
        # engine memsets (each startup DMA costs ~630ns of HWDGE queue time
        # regardless of size)
        nc.sync.dma_start(const_t[:, 0:CSPLIT], CONST_d[:, 0:CSPLIT])
        nc.sync.dma_start(const_t[:, CSPLIT:], CONST_d[:, CSPLIT:])
        for kc in range(2):
            nc.vector.memset(PA[kc][:].bitcast(dt.float32), 0.0)
            nc.vector.memset(PB[kc][:].bitcast(dt.float32), 0.0)
        for i in range(GPF):
            nc.gpsimd.memset(Gr[i][0][:], 0.0)
            nc.gpsimd.memset(Gr[i][1][:], 0.0)
        nc.vector.memset(meas_t[:], 0.0)

        def g_dma(j):
            """DMA G[j] (BB, 96, 96) into ring slot j%GPF (central rows)."""
            for kc in range(2):
                gt = Gr[j % GPF][kc]
                plo = COFF if kc == 0 else 0          # central partition base
                rlo = 0 if kc == 0 else 48            # central row base
                src = G_d[j, :, rlo:rlo + 48, :].rearrange("f r c -> r f c")
                dst = gt[plo:plo + 48, :].rearrange("p (f c) -> p f c", c=CW)
                # Activation's HWDGE queue: keeps the G stream off the SP
                # queue so startup consts and G prefetch flow in parallel
                nc.scalar.dma_start(dst, src)

        def g_presub(j, prev):
            """prev (old state, p0) -= G[j] on the central cols (GpSimd)."""
            for kc in range(2):
                base = prev[kc][:, PAD:PAD + CHW]

                def cvi(b):
                    return b.rearrange("p (f c) -> p f c", c=SEG)[
                        :, :, 2 + COFF:2 + COFF + CW]
                gv = Gr[j % GPF][kc][:].rearrange("p (f c) -> p f c", c=CW)
                nc.gpsimd.tensor_sub(
                    cvi(base), cvi(base.bitcast(dt.float32)), gv)

        # matmul operands use the strided 2-field data view: free size
        # 2x142 = 284 still clears the fp32r N>=256 threshold, so each
        # matmul streams 284 columns instead of a contiguous 288 window
        # (cost model prices output free-size; guards are never written)
        def run_view(t, off=0):
            """Strided [71, 2, 142] matmul-rhs data view at col-tap off."""
            v = t[:, PAD + off: PAD + off + CHW]
            return v.rearrange("p (f c) -> p f c", c=SEG)[:, :, 2:2 + D]

        def data_view(t, cast_f32=False):
            """[71, 2(field), 142] data view (for DVE ops)."""
            v = t[:, PAD:PAD + CHW]
            if cast_f32:
                v = v.bitcast(dt.float32)
            return v.rearrange("p (f c) -> p f c", c=SEG)[:, :, 2:2 + D]

        def central_view(t, cast_f32=False):
            """[71, 2(field), 96] central-cols view of a state chunk tile."""
            v = t[:, PAD:PAD + CHW]
            if cast_f32:
                v = v.bitcast(dt.float32)
            return v.rearrange("p (f c) -> p f c", c=SEG)[
                :, :, 2 + COFF:2 + COFF + CW]

        def meas_extract(pt, j, direct=False):
            """Extract 32x2 measurements for output step j from selection rows.

            Act stages the PSUM aug rows into SBUF (keeps the PSUM read off
            DVE); the per-field mask-reduce STTs then run on DVE. The
            epilogue uses direct=True (STT straight from PSUM) since the
            staging hop only adds serial latency there."""
            if direct:
                for f in range(2):
                    seg = pt[96:96 + NMEAS, f * SEG + 2: f * SEG + 2 + D]
                    nc.vector.scalar_tensor_tensor(
                        out=scr_t[f][:], in0=seg, scalar=1.0, in1=mask_t,
                        op0=mybir.AluOpType.mult, op1=mybir.AluOpType.mult,
                        accum_out=meas_t[:, f * NT + j: f * NT + j + 1],
                    )
                return
            at = augs_t[j % 2]
            src = pt[96:96 + NMEAS, :].rearrange(
                "p (f c) -> p f c", c=SEG)[:, :, 2:2 + D]
            nc.scalar.copy(at[:].rearrange("p (f c) -> p f c", c=D), src)
            for f, eng in ((0, nc.vector), (1, nc.vector)):
                eng.scalar_tensor_tensor(
                    out=scr_t[f][:], in0=at[:, f * D:(f + 1) * D], scalar=1.0,
                    in1=mask_t,
                    op0=mybir.AluOpType.mult, op1=mybir.AluOpType.mult,
                    accum_out=meas_t[:, f * NT + j: f * NT + j + 1],
                )

        cur, prev = PA, PB
        for rep in range(reps):
          if rep > 0:
            # re-zero state so values stay bounded across timing reps
            for kc in range(2):
                nc.vector.memset(PA[kc][:].bitcast(dt.float32), 0.0)
                nc.vector.memset(PB[kc][:].bitcast(dt.float32), 0.0)
          # iters 0,1 are skipped: G[0]=G[1]=0 (host zeroes d2[:, :2]) and
          # the state starts at zero, so p_1 = p_2 = 0 and output cols 0,1
          # are exactly zero -- provided by the meas_t memset.
          J0 = 2
          for j in range(J0, nt):
              if j == J0:
                  for q in range(J0, min(J0 + GPF - 1, nt)):
                      g_dma(q)
              if j + GPF - 1 < nt:
                  g_dma(j + GPF - 1)

              g_presub(j, prev)

              # emission order: all cur[0]-gated matmuls first (ps0 body +
              # ps1's bd01 start), then the cur[1]-gated tail; bd10 closes
              # ps0 as late as its dependency allows so copyback0 tracks
              # copyback1's phase.
              pt0 = ps_pool.tile([128, CHW], dt.float32, tag="ps0")
              pt1 = ps_pool.tile([K, CHW], dt.float32, tag="ps1")

              def dv(ap):
                  return ap.rearrange("p (f c) -> p f c", c=SEG)[:, :, 2:2 + D]
              pd0, pd1 = dv(pt0[0:K, :]), dv(pt1[0:K, :])
              full0 = dv(pt0[:])
              nc.tensor.matmul(full0, bd_t[(0, 0)], run_view(cur[0]),
                               start=True, stop=False)
              for s, sh in ((-1, sh1_t), (1, sh1_t), (-2, sh2_t), (2, sh2_t)):
                  nc.tensor.matmul(pd0, sh, run_view(cur[0], s),
                                   start=False, stop=False)
              nc.tensor.matmul(pd1, bd_t[(0, 1)], run_view(cur[0]),
                               start=True, stop=False)
              nc.tensor.matmul(full0, bd_t[(1, 0)], run_view(cur[1]),
                               start=False, stop=True)
              for s, sh in ((-1, sh1_t), (1, sh1_t), (-2, sh2_t), (2, sh2_t)):
                  nc.tensor.matmul(pd1, sh, run_view(cur[1], s),
                                   start=False, stop=False)
              nc.tensor.matmul(pd1, bd_t[(1, 1)], run_view(cur[1]),
                               start=False, stop=True)
              psums = [pt0, pt1]

              for mc in range(2):
                  pd = psums[mc][0:K, :].rearrange(
                      "p (f c) -> p f c", c=SEG)[:, :, 2:2 + D]
                  nc.vector.tensor_tensor(
                      out=data_view(prev[mc]), in0=pd,
                      in1=data_view(prev[mc], cast_f32=True),
                      op=mybir.AluOpType.subtract)

              if j > 0:
                  meas_extract(psums[0], j - 1, direct=(j == nt - 1))

              cur, prev = prev, cur

        # final measurement for output step nt-1 on the final state
        pt = ps_pool.tile([128, CHW], dt.float32, tag="ps0")
        fdv = pt[:].rearrange("p (f c) -> p f c", c=SEG)[:, :, 2:2 + D]
        nc.tensor.matmul(fdv, bd_t[(0, 0)], run_view(cur[0]),
                         start=True, stop=False)
        nc.tensor.matmul(fdv, bd_t[(1, 0)], run_view(cur[1]),
                         start=False, stop=True)
        meas_extract(pt, nt - 1, direct=True)

        if debug:
            for kc in range(2):
                nc.sync.dma_start(DBGC_d[kc], cur[kc][:].bitcast(dt.float32))
                nc.sync.dma_start(DBGP_d[kc], prev[kc][:].bitcast(dt.float32))
        nc.sync.dma_start(
            OUT_d[:].rearrange("f i j -> i f j"),
            meas_t[:].rearrange("i (f j) -> i f j", j=NT))

    nc.compile()
    return nc


def kernel(x, P0):
    x = np.asarray(x, dtype=np.float32)
    P0 = np.asarray(P0, dtype=np.float32)
    from concourse.bass_utils import run_bass_kernel_spmd

    if "prog" not in _prog_cache:
        _prog_cache["prog"] = _build_program()
    nc = _prog_cache["prog"]

    BD, SH1, SH2, MASK = _build_band_consts()

    xx = bg / x[:, 0]
    rf = (1.0 - xx * xx).astype(np.float32)           # (B, 96, 96)
    P0c = P0[0, :, :, CLO:CHI, CLO:CHI]               # (NR, NT, 96, 96)
    d2 = np.zeros_like(P0c)
    d2[:, 2:] = P0c[:, 2:] - 2.0 * P0c[:, 1:-1] + P0c[:, :-2]

    # pack constants in the kernel's cslice order:
    # bd00[128] bd10[128] bd01[71] sh1[71] sh2[71] | bd11[71] mask[142]
    CPACK = 128 + 128 + K + K + K + K + D
    cp = np.zeros((K, CPACK), np.float32)
    off = 0
    for blk in (BD[(0, 0)], BD[(1, 0)], BD[(0, 1)], SH1, SH2, BD[(1, 1)]):
        w = blk.shape[1]
        cp[:, off:off + w] = blk
        off += w
    cp[:NMEAS, off:off + D] = MASK
    consts = {"CONSTS": cp}

    in_maps = []
    for r in range(NRR):
        G = (rf[None, :, :, :] * d2[r][:, None, :, :]).astype(np.float32)
        m = dict(consts)
        m["G"] = np.ascontiguousarray(G)
        in_maps.append(m)

    trace = bool(int(os.environ.get("KERNEL_TRACE", "0")))
    res = run_bass_kernel_spmd(nc, in_maps, core_ids=list(range(NRR)),
                               trace=trace)
    _prog_cache["last_result"] = res
    out = np.zeros((BB, NRR, NMEAS, NT), np.float32)
    for r in range(NRR):
        out[:, r] = res.results[r]["OUT"]
    return out



# revision 59
# speedup vs baseline: 1.0560x; 1.0000x over previous
"""Trainium2 Bass kernel for nn_BornFoward: 200-step leapfrog wave recurrence.

Math (validated against the jax reference in a numpy model):
  - coef = (dt*BGF/dx)^2 is 0.2025 in the interior square [25:167)^2 of the
    192x192 grid and ~4.4e-13 in the outer absorbing ring; rf is EXACTLY zero
    outside the central 96x96 window (pad region has X==1 -> 1-X^2==0).
  - Therefore the recurrence restricted to the 142x142 interior with zero
    Dirichlet boundary and constant coef reproduces the reference to ~1e-9.
  - p_new = 2*p1 - p0 + C*lap4(p1) + rf*d2(P0),  meas = p_new at 32 pixels.

Sharding: 16 independent recurrences (B=2 x NR=8) -> channel r per core,
both batches per core, batched along the matmul free (column) dimension.

Layout per core: state tiles [71 partitions, 2 chunks x 292], where each
chunk holds rows 71k..71k+70 as two field segments [2 guard | 142 | 2 guard].
All matmul rhs/out operands are contiguous 288-column windows ([2,290) of
the run; N=288 >= 256 so float32r matmuls stream at 1 cycle/row, ~120ns).

Per-core per-step compute (engine assignment balances the 5 engines):
  PE (12 matmuls):
    PSUM_m = band_x @ p1        (x-stencil + diag + 2I; 2 K-chunks; +32 meas
                                 selection rows augmented onto chunk-0 lhsT)
           + a*I @ p1(cols+-1) + b*I @ p1(cols+-2)  (y-stencil, shifted rhs)
  GpSimd:  p0 -= G[j]           (presubtract on the central cols; p0 is dead
                                 as a matmul operand -> a full step of slack)
  DVE:     p_new = PSUM_m - p0  (fused copyback, rotates state tiles)
  Act:     stages the 32 PSUM selection rows into SBUF
  DVE:     per-field one-hot mask-reduce STTs (SBUF-sourced, 208ns each)

The G term enters via p_new = PSUM - (p0 - G), eliminating 2 of the 14
matmuls per step. Steady state ~1715ns/step: bounded not by PE occupancy
(84%) but by the recurrence-carried chain  ps-close -> sem -> DVE copyback
(421ns) -> ack+sem -> next step's matmuls  (~924ns of which ~520 is fixed
sem/ack latency). Only DVE can do PSUM-elementwise ops, matmul rhs must be
SBUF, and engine free-size costing makes tiny-partition fix-ups cost as
much as full ones, so this chain is structural; see session notes.

Startup: one packed-constants DMA + engine memsets (each DMA costs ~630ns
of HWDGE queue time regardless of size), first matmul at ~3.4us.
"""
import sys
import os
import numpy as np
from contextlib import ExitStack

sys.path.insert(0, "/opt/trn_rl_repo")

# ---- problem constants (hardcoded; kernel.py must be self-contained) ----
NX = 192
NT = 200
dtime = 0.3
nm, sR = 32, 70
bg = 1.5
LO, HI = 25, 167            # interior rows/cols [LO, HI) -> D = 142
D = HI - LO
CLO, CHI = 48, 144          # central rf-support window (96 wide)
CW = CHI - CLO
COFF = CLO - LO             # 23: central window offset inside domain
C = (dtime * bg / 1.0) ** 2  # 0.2025
K = 71                      # row-chunk size (2 chunks of 71 = 142)
SEG = 2 + D + 2             # 146: per-field segment with 2-col guards
CHW = 2 * SEG               # 292: chunk width (two fields)
NRR = 8
BB = 2
NMEAS = nm

_thetas = 2 * np.pi * np.arange(nm) / nm
_MX = (NX / 2 + sR * np.cos(_thetas)).astype(int)
_MY = (NX / 2 + sR * np.sin(_thetas)).astype(int)

INCLUDE_2I = True           # fold the 2*p1 term into the band matmul

_prog_cache = {}


def _build_band_consts():
    """Host-side constant matrices for the matmuls (numpy float32)."""
    S = np.zeros((D, D), np.float32)
    idx = np.arange(D)
    S[idx, idx] = -60.0 * C / 12.0 + (2.0 if INCLUDE_2I else 0.0)
    S[idx[:-1], idx[:-1] + 1] = 16.0 * C / 12.0
    S[idx[1:], idx[1:] - 1] = 16.0 * C / 12.0
    S[idx[:-2], idx[:-2] + 2] = -C / 12.0
    S[idx[2:], idx[2:] - 2] = -C / 12.0

    BD = {}
    for kc in range(2):
        for mc in range(2):
            blk = S[mc * K:(mc + 1) * K, kc * K:(kc + 1) * K].T.copy()
            if mc == 0:
                aug = np.zeros((K, 96 + NMEAS), np.float32)
                aug[:, :K] = blk
                for i in range(NMEAS):
                    g = _MX[i] - LO
                    if g // K == kc:
                        aug[g % K, 96 + i] = 1.0
                blk = aug
            BD[(kc, mc)] = np.ascontiguousarray(blk)

    SH1 = np.eye(K, dtype=np.float32) * np.float32(16.0 * C / 12.0)
    SH2 = np.eye(K, dtype=np.float32) * np.float32(-C / 12.0)

    # per-field one-hot masks over the 142 data cols
    MASK = np.zeros((NMEAS, D), np.float32)
    for i in range(NMEAS):
        MASK[i, _MY[i] - LO] = 1.0
    return BD, SH1, SH2, MASK


def _build_program(nt=NT, debug=False, reps=1):
    import concourse.bacc as bacc
    import concourse.tile as tile
    import concourse.mybir as mybir

    dt = mybir.dt
    nc = bacc.Bacc("TRN2", target_bir_lowering=False)

    G_d = nc.dram_tensor("G", (NT, BB, CW, CW), dt.float32, kind="ExternalInput")
    # all small constants packed into one tensor -> single startup DMA
    # layout (cols): bd00[128] bd01[128] bd10[71] bd11[71] sh1[71] sh2[71]
    #                mask[142] = 682
    CPACK = 128 + 128 + K + K + K + K + D
    CONST_d = nc.dram_tensor("CONSTS", (K, CPACK), dt.float32r,
                             kind="ExternalInput")
    OUT_d = nc.dram_tensor("OUT", (BB, NMEAS, NT), dt.float32, kind="ExternalOutput")
    if debug:
        DBGC_d = nc.dram_tensor("DBGC", (2, K, 300), dt.float32, kind="ExternalOutput")
        DBGP_d = nc.dram_tensor("DBGP", (2, K, 300), dt.float32, kind="ExternalOutput")

    GPF = 4  # G stream ring depth
    PAD = 4  # left/right pad so shift offsets stay in-bounds

    with tile.TileContext(nc) as tc, ExitStack() as ctx:
        def sbuf(name, shape, dty):
            return ctx.enter_context(nc.sbuf_tensor(name, shape, dty))

        # per-chunk state tiles: [4 pad | 2 x (2+142+2) | 4 pad] = 300 cols
        PA = [sbuf(f"PA{kc}", [K, 300], dt.float32r) for kc in range(2)]
        PB = [sbuf(f"PB{kc}", [K, 300], dt.float32r) for kc in range(2)]
        # G ring: compact central-cols tiles per chunk; partitions outside
        # the central rows stay zero (engine ops need 32-aligned partition
        # bases, so the presub spans all 71 chunk rows)
        Gr = [[sbuf(f"Gr{i}_{kc}", [K, BB * CW], dt.float32) for kc in range(2)]
              for i in range(GPF)]
        const_t = sbuf("consts", [K, CPACK], dt.float32r)
        _off = [0]

        def cslice(w, parts=K, cast=None):
            lo = _off[0]
            _off[0] += w
            v = const_t[0:parts, lo:lo + w]
            return v.bitcast(cast) if cast is not None else v

        # pack order groups the consts needed by the first 7 matmul slots
        # (bd00, bd10, bd01, sh1, sh2) ahead of the late ones (bd11, mask)
        # so the startup DMA can split and un-gate the first step earlier
        bd_t = {(0, 0): cslice(128), (1, 0): cslice(128), (0, 1): cslice(K)}
        sh1_t = cslice(K)
        sh2_t = cslice(K)
        CSPLIT = _off[0]                                  # 469
        bd_t[(1, 1)] = cslice(K)
        mask_t = cslice(D, parts=NMEAS, cast=dt.float32)
        meas_t = sbuf("meas", [NMEAS, BB * NT], dt.float32)
        scr_t = [sbuf(f"scr{f}", [NMEAS, D], dt.float32) for f in range(2)]
        # staging for the PSUM aug rows (Act copies here, STTs read SBUF)
        augs_t = [sbuf(f"augs{i}", [NMEAS, 2 * D], dt.float32)
                  for i in range(3)]

        ps_pool = ctx.enter_context(tc.tile_pool(name="ps", bufs=4, space="PSUM"))

        # constants in two DMAs (early-gating slice first); zero-inits via
        # engine memsets (each startup DMA costs ~630ns of HWDGE queue time
        # regardless of size)
        nc.sync.dma_start(const_t[:, 0:CSPLIT], CONST_d[:, 0:CSPLIT])
        nc.sync.dma_start(const_t[:, CSPLIT:], CONST_d[:, CSPLIT:])
        for kc in range(2):
            nc.vector.memset(PA[kc][:].bitcast(dt.float32), 0.0)
            nc.vector.memset(PB[kc][:].bitcast(dt.float32), 0.0)
        for i in range(GPF):
            nc.gpsimd.memset(Gr[i][0][:], 0.0)
            nc.gpsimd.memset(Gr[i][1][:], 0.0)
        nc.vector.memset(meas_t[:], 0.0)

        def g_dma(j):
            """DMA G[j] (BB, 96, 96) into ring slot j%GPF (central rows)."""
            for kc in range(2):
                gt = Gr[j % GPF][kc]
                plo = COFF if kc == 0 else 0          # central partition base
                rlo = 0 if kc == 0 else 48            # central row base
                src = G_d[j, :, rlo:rlo + 48, :].rearrange("f r c -> r f c")
                dst = gt[plo:plo + 48, :].rearrange("p (f c) -> p f c", c=CW)
                # Activation's HWDGE queue: keeps the G stream off the SP
                # queue so startup consts and G prefetch flow in parallel
                nc.scalar.dma_start(dst, src)

        def g_presub(j, prev):
            """prev (old state, p0) -= G[j] on the central cols (GpSimd)."""
            for kc in range(2):
                base = prev[kc][:, PAD:PAD + CHW]

                def cvi(b):
                    return b.rearrange("p (f c) -> p f c", c=SEG)[
                        :, :, 2 + COFF:2 + COFF + CW]
                gv = Gr[j % GPF][kc][:].rearrange("p (f c) -> p f c", c=CW)
                nc.gpsimd.tensor_sub(
                    cvi(base), cvi(base.bitcast(dt.float32)), gv)

        # matmul operands use the strided 2-field data view: free size
        # 2x142 = 284 still clears the fp32r N>=256 threshold, so each
        # matmul streams 284 columns instead of a contiguous 288 window
        # (cost model prices output free-size; guards are never written)
        def run_view(t, off=0):
            """Strided [71, 2, 142] matmul-rhs data view at col-tap off."""
            v = t[:, PAD + off: PAD + off + CHW]
            return v.rearrange("p (f c) -> p f c", c=SEG)[:, :, 2:2 + D]

        def data_view(t, cast_f32=False):
            """[71, 2(field), 142] data view (for DVE ops)."""
            v = t[:, PAD:PAD + CHW]
            if cast_f32:
                v = v.bitcast(dt.float32)
            return v.rearrange("p (f c) -> p f c", c=SEG)[:, :, 2:2 + D]

        def central_view(t, cast_f32=False):
            """[71, 2(field), 96] central-cols view of a state chunk tile."""
            v = t[:, PAD:PAD + CHW]
            if cast_f32:
                v = v.bitcast(dt.float32)
            return v.rearrange("p (f c) -> p f c", c=SEG)[
                :, :, 2 + COFF:2 + COFF + CW]

        def meas_extract(pt, j, direct=False):
            """Extract 32x2 measurements for output step j from selection rows.

            Act stages the PSUM aug rows into SBUF (keeps the PSUM read off
            DVE); the per-field mask-reduce STTs then run on DVE. The
            epilogue uses direct=True (STT straight from PSUM) since the
            staging hop only adds serial latency there."""
            if direct:
                for f in range(2):
                    seg = pt[96:96 + NMEAS, f * SEG + 2: f * SEG + 2 + D]
                    nc.vector.scalar_tensor_tensor(
                        out=scr_t[f][:], in0=seg, scalar=1.0, in1=mask_t,
                        op0=mybir.AluOpType.mult, op1=mybir.AluOpType.mult,
                        accum_out=meas_t[:, f * NT + j: f * NT + j + 1],
                    )
                return
            at = augs_t[j % 3]
            src = pt[96:96 + NMEAS, :].rearrange(
                "p (f c) -> p f c", c=SEG)[:, :, 2:2 + D]
            nc.scalar.copy(at[:].rearrange("p (f c) -> p f c", c=D), src)
            for f, eng in ((0, nc.vector), (1, nc.vector)):
                eng.scalar_tensor_tensor(
                    out=scr_t[f][:], in0=at[:, f * D:(f + 1) * D], scalar=1.0,
                    in1=mask_t,
                    op0=mybir.AluOpType.mult, op1=mybir.AluOpType.mult,
                    accum_out=meas_t[:, f * NT + j: f * NT + j + 1],
                )

        cur, prev = PA, PB
        for rep in range(reps):
          if rep > 0:
            # re-zero state so values stay bounded across timing reps
            for kc in range(2):
                nc.vector.memset(PA[kc][:].bitcast(dt.float32), 0.0)
                nc.vector.memset(PB[kc][:].bitcast(dt.float32), 0.0)
          # iters 0,1 are skipped: G[0]=G[1]=0 (host zeroes d2[:, :2]) and
          # the state starts at zero, so p_1 = p_2 = 0 and output cols 0,1
          # are exactly zero -- provided by the meas_t memset.
          J0 = 2
          for j in range(J0, nt):
              if j == J0:
                  for q in range(J0, min(J0 + GPF - 1, nt)):
                      g_dma(q)
              if j + GPF - 1 < nt:
                  g_dma(j + GPF - 1)

              # emission order: all cur[0]-gated matmuls first (ps0 body +
              # ps1's bd01 start), then the cur[1]-gated tail; bd10 closes
              # ps0 as late as its dependency allows so copyback0 tracks
              # copyback1's phase.
              pt0 = ps_pool.tile([128, CHW], dt.float32, tag="ps0")
              pt1 = ps_pool.tile([K, CHW], dt.float32, tag="ps1")

              def dv(ap):
                  return ap.rearrange("p (f c) -> p f c", c=SEG)[:, :, 2:2 + D]
              pd0, pd1 = dv(pt0[0:K, :]), dv(pt1[0:K, :])
              full0 = dv(pt0[:])
              nc.tensor.matmul(full0, bd_t[(0, 0)], run_view(cur[0]),
                               start=True, stop=False)
              for s, sh in ((-1, sh1_t), (1, sh1_t), (-2, sh2_t), (2, sh2_t)):
                  nc.tensor.matmul(pd0, sh, run_view(cur[0], s),
                                   start=False, stop=False)
              nc.tensor.matmul(pd1, bd_t[(0, 1)], run_view(cur[0]),
                               start=True, stop=False)
              nc.tensor.matmul(full0, bd_t[(1, 0)], run_view(cur[1]),
                               start=False, stop=True)
              for s, sh in ((-1, sh1_t), (1, sh1_t), (-2, sh2_t), (2, sh2_t)):
                  nc.tensor.matmul(pd1, sh, run_view(cur[1], s),
                                   start=False, stop=False)
              nc.tensor.matmul(pd1, bd_t[(1, 1)], run_view(cur[1]),
                               start=False, stop=True)
              psums = [pt0, pt1]

              g_presub(j, prev)

              for mc in range(2):
                  pd = psums[mc][0:K, :].rearrange(
                      "p (f c) -> p f c", c=SEG)[:, :, 2:2 + D]
                  nc.vector.tensor_tensor(
                      out=data_view(prev[mc]), in0=pd,
                      in1=data_view(prev[mc], cast_f32=True),
                      op=mybir.AluOpType.subtract)

              if j > 0:
                  meas_extract(psums[0], j - 1, direct=(j == nt - 1))

              cur, prev = prev, cur

        # final measurement for output step nt-1 on the final state
        pt = ps_pool.tile([128, CHW], dt.float32, tag="ps0")
        fdv = pt[:].rearrange("p (f c) -> p f c", c=SEG)[:, :, 2:2 + D]
        nc.tensor.matmul(fdv, bd_t[(0, 0)], run_view(cur[0]),
                         start=True, stop=False)
        nc.tensor.matmul(fdv, bd_t[(1, 0)], run_view(cur[1]),
                         start=False, stop=True)
        meas_extract(pt, nt - 1, direct=True)

        if debug:
            for kc in range(2):
                nc.sync.dma_start(DBGC_d[kc], cur[kc][:].bitcast(dt.float32))
                nc.sync.dma_start(DBGP_d[kc], prev[kc][:].bitcast(dt.float32))
        nc.sync.dma_start(
            OUT_d[:].rearrange("f i j -> i f j"),
            meas_t[:].rearrange("i (f j) -> i f j", j=NT))

    nc.compile()
    return nc


def kernel(x, P0):
    x = np.asarray(x, dtype=np.float32)
    P0 = np.asarray(P0, dtype=np.float32)
    from concourse.bass_utils import run_bass_kernel_spmd

    if "prog" not in _prog_cache:
        _prog_cache["prog"] = _build_program()
    nc = _prog_cache["prog"]

    BD, SH1, SH2, MASK = _build_band_consts()

    xx = bg / x[:, 0]
    rf = (1.0 - xx * xx).astype(np.float32)           # (B, 96, 96)
    P0c = P0[0, :, :, CLO:CHI, CLO:CHI]               # (NR, NT, 96, 96)
    d2 = np.zeros_like(P0c)
    d2[:, 2:] = P0c[:, 2:] - 2.0 * P0c[:, 1:-1] + P0c[:, :-2]

    # pack constants in the kernel's cslice order:
    # bd00[128] bd10[128] bd01[71] sh1[71] sh2[71] | bd11[71] mask[142]
    CPACK = 128 + 128 + K + K + K + K + D
    cp = np.zeros((K, CPACK), np.float32)
    off = 0
    for blk in (BD[(0, 0)], BD[(1, 0)], BD[(0, 1)], SH1, SH2, BD[(1, 1)]):
        w = blk.shape[1]
        cp[:, off:off + w] = blk
        off += w
    cp[:NMEAS, off:off + D] = MASK
    consts = {"CONSTS": cp}

    in_maps = []
    for r in range(NRR):
        G = (rf[None, :, :, :] * d2[r][:, None, :, :]).astype(np.float32)
        m = dict(consts)
        m["G"] = np.ascontiguousarray(G)
        in_maps.append(m)

    trace = bool(int(os.environ.get("KERNEL_TRACE", "0")))
    res = run_bass_kernel_spmd(nc, in_maps, core_ids=list(range(NRR)),
                               trace=trace)
    _prog_cache["last_result"] = res
    out = np.zeros((BB, NRR, NMEAS, NT), np.float32)
    for r in range(NRR):
        out[:, r] = res.results[r]["OUT"]
    return out

